# revision 1
# baseline (speedup 1.0000x reference)
"""Trainium2 Bass kernel for the 9-layer dense MLP (dropout-mask training forward).

Strategy (pure data parallel, 8 cores, 8192 batch rows each):
  - Activations kept transposed on-chip: features on partitions, batch cols on free dim.
    Each layer computes zT = W^T @ hT via nc.tensor.matmul(out, lhsT=W, rhs=hT).
  - fp16 weights/activations/masks (fp32 PSUM accumulation), fp32 biases + output.
  - Dropout masks binarized on host ({0,1} fp16); the 1/keep scale is folded into the
    next layer's weights.
  - Per batch tile of 512 columns: matmul chain L1..L9 with PSUM accumulation over
    K-chunks; PSUM drained with fused bias+relu (ScalarE activation or DVE
    tensor_scalar); mask multiply on DVE/GpSimd.
  - One packed-input DMA per tile (x chunks + all mask chunks interleaved host-side).
  - Layers 6/7/8 are partition-packed (outputs at partition offsets 0/64/96 via
    matmul tile_position) so their masks ride in one packed chunk.
"""

import sys

sys.path.insert(0, "/opt/trn_rl_repo")

import numpy as np

DIMS = [256, 128, 256, 512, 256, 128, 64, 32, 16, 10]
NCORES = 8
BATCH = 65536
SHARD = BATCH // NCORES  # 8192
MTILE = 512
NTILES = SHARD // MTILE  # 16

# pack chunk layout (each chunk = 128 partitions x 8192 cols, fp16):
#   0,1: xT        2: m1        3,4: m2      5-8: m3
#   9,10: m4       11: m5       12: m6/m7/m8 partition-packed at rows 0/64/96
NPACK = 13

_PROG = {}


def _build_program():
    import concourse.bass as bass
    import concourse.tile as tile
    from concourse import bacc, mybir

    f16 = mybir.dt.float16
    f32 = mybir.dt.float32
    RELU = mybir.ActivationFunctionType.Relu
    IDENT = mybir.ActivationFunctionType.Identity
    ADD = mybir.AluOpType.add
    MAX = mybir.AluOpType.max

    nc = bacc.Bacc("TRN2", target_bir_lowering=False, debug=False, num_devices=NCORES)

    pack_d = nc.dram_tensor("pack", [128, NPACK, SHARD], f16, kind="ExternalInput").ap()
    # lhsT-packed weights: (partitions, Kc, N) with K-chunk k at [:, k, :]
    w_d = {}
    w_d[1] = nc.dram_tensor("W1", [128, 2, 128], f16, kind="ExternalInput").ap()
    w_d[2] = nc.dram_tensor("W2", [128, 1, 256], f16, kind="ExternalInput").ap()
    w_d[3] = nc.dram_tensor("W3", [128, 2, 512], f16, kind="ExternalInput").ap()
    w_d[4] = nc.dram_tensor("W4", [128, 4, 256], f16, kind="ExternalInput").ap()
    w_d[5] = nc.dram_tensor("W5", [128, 2, 128], f16, kind="ExternalInput").ap()
    w_d[6] = nc.dram_tensor("W6", [128, 1, 64], f16, kind="ExternalInput").ap()
    w_d[7] = nc.dram_tensor("W7", [64, 32], f16, kind="ExternalInput").ap()
    w_d[8] = nc.dram_tensor("W8", [32, 16], f16, kind="ExternalInput").ap()
    w_d[9] = nc.dram_tensor("W9", [16, 10], f16, kind="ExternalInput").ap()
    b15_d = nc.dram_tensor("B15", [128, 10], f32, kind="ExternalInput").ap()
    b678_d = nc.dram_tensor("B678", [128, 1], f32, kind="ExternalInput").ap()
    b9_d = nc.dram_tensor("B9", [10, 1], f32, kind="ExternalInput").ap()
    out_d = nc.dram_tensor("outT", [10, SHARD], f32, kind="ExternalOutput").ap()

    with tile.TileContext(nc) as tc:
        with (
            tc.tile_pool(name="wpool", bufs=1) as wp,
            tc.tile_pool(name="pack", bufs=3) as packp,
            tc.tile_pool(name="hr", bufs=2) as hrp,
            tc.tile_pool(name="hm", bufs=2) as hmp,
            tc.tile_pool(name="osb", bufs=3) as outp,
            tc.tile_pool(name="psS", bufs=3, space="PSUM") as psS,
            tc.tile_pool(name="psM", bufs=2, space="PSUM") as psM,
        ):
            w1 = wp.tile([128, 2, 128], f16, tag="w1")
            w2 = wp.tile([128, 1, 256], f16, tag="w2")
            w3 = wp.tile([128, 2, 512], f16, tag="w3")
            w4 = wp.tile([128, 4, 256], f16, tag="w4")
            w5 = wp.tile([128, 2, 128], f16, tag="w5")
            w6 = wp.tile([128, 1, 64], f16, tag="w6")
            w789 = wp.tile([128, 64], f16, tag="w789")  # W7 @[0:64,0:32], W8 @[64:96,32:48], W9 @[96:112,48:58]
            b15 = wp.tile([128, 10], f32, tag="b15")
            b678 = wp.tile([128, 1], f32, tag="b678")
            b9 = wp.tile([10, 1], f32, tag="b9")
            for t_, d_ in ((w1, w_d[1]), (w2, w_d[2]), (w3, w_d[3]), (w4, w_d[4]),
                           (w5, w_d[5]), (w6, w_d[6]), (b15, b15_d), (b678, b678_d), (b9, b9_d)):
                nc.sync.dma_start(t_[:], d_[:])
            nc.sync.dma_start(w789[0:64, 0:32], w_d[7][:])
            nc.sync.dma_start(w789[64:96, 32:48], w_d[8][:])
            nc.sync.dma_start(w789[96:112, 48:58], w_d[9][:])

            for t in range(NTILES):
                cs = bass.ts(t, MTILE)
                pk = packp.tile([128, NPACK, MTILE], f16, tag="pk")
                nc.sync.dma_start(pk[:], pack_d[:, :, cs])

                # ---- L1: 256 -> 128 (mask m1 = pk[:,2]) ----
                z1 = psS.tile([128, MTILE], f32, tag="s")
                nc.tensor.matmul(z1[:], w1[:, 0, :], pk[:, 0, :], start=True, stop=False)
                nc.tensor.matmul(z1[:], w1[:, 1, :], pk[:, 1, :], start=False, stop=True)
                hr1 = hrp.tile([128, MTILE], f16, tag="hr1")
                nc.vector.tensor_scalar(hr1[:], z1[:], b15[:, 0:1], 0.0, ADD, MAX)
                hm1 = hmp.tile([128, MTILE], f16, tag="hm1")
                nc.gpsimd.tensor_mul(hm1[:], hr1[:], pk[:, 2, :])

                # ---- L2: 128 -> 256 (m2 = pk[:,3:5]) ----
                z2 = psM.tile([128, 2, MTILE], f32, tag="m")
                for c in range(2):
                    nc.tensor.matmul(z2[:, c, :], w2[:, 0, bass.ts(c, 128)], hm1[:],
                                     start=True, stop=True)
                hr2 = hrp.tile([128, 2, MTILE], f16, tag="hr2")
                for c in range(2):
                    nc.scalar.activation(hr2[:, c, :], z2[:, c, :], RELU, bias=b15[:, 1 + c:2 + c])
                hm2 = hmp.tile([128, 2, MTILE], f16, tag="hm2")
                nc.vector.tensor_mul(hm2[:], hr2[:], pk[:, 3:5, :])

                # ---- L3: 256 -> 512 (m3 = pk[:,5:9]) ----
                z3a = psM.tile([128, 2, MTILE], f32, tag="m")
                z3b = psM.tile([128, 2, MTILE], f32, tag="m")
                hr3 = hrp.tile([128, 4, MTILE], f16, tag="hr3")
                for c in range(4):
                    zt = z3a if c < 2 else z3b
                    for k in range(2):
                        nc.tensor.matmul(zt[:, c % 2, :], w3[:, k, bass.ts(c, 128)],
                                         hm2[:, k, :], start=(k == 0), stop=(k == 1))
                    nc.scalar.activation(hr3[:, c, :], zt[:, c % 2, :], RELU, bias=b15[:, 3 + c:4 + c])
                hm3 = hmp.tile([128, 4, MTILE], f16, tag="hm3")
                nc.vector.tensor_mul(hm3[:], hr3[:], pk[:, 5:9, :])

                # ---- L4: 512 -> 256 (m4 = pk[:,9:11]) ----
                z4 = psM.tile([128, 2, MTILE], f32, tag="m")
                hr4 = hrp.tile([128, 2, MTILE], f16, tag="hr4")
                for c in range(2):
                    for k in range(4):
                        nc.tensor.matmul(z4[:, c, :], w4[:, k, bass.ts(c, 128)],
                                         hm3[:, k, :], start=(k == 0), stop=(k == 3))
                    nc.scalar.activation(hr4[:, c, :], z4[:, c, :], RELU, bias=b15[:, 7 + c:8 + c])
                hm4 = hmp.tile([128, 2, MTILE], f16, tag="hm4")
                nc.vector.tensor_mul(hm4[:], hr4[:], pk[:, 9:11, :])

                # ---- L5: 256 -> 128 (m5 = pk[:,11]) ----
                z5 = psS.tile([128, MTILE], f32, tag="s")
                for k in range(2):
                    nc.tensor.matmul(z5[:], w5[:, k, :], hm4[:, k, :], start=(k == 0), stop=(k == 1))
                hr5 = hrp.tile([128, MTILE], f16, tag="hr5")
                nc.vector.tensor_scalar(hr5[:], z5[:], b15[:, 9:10], 0.0, ADD, MAX)
                hm5 = hmp.tile([128, MTILE], f16, tag="hm5")
                nc.gpsimd.tensor_mul(hm5[:], hr5[:], pk[:, 11, :])

                # ---- L6/L7/L8: partition-packed at 0/64/96 (m678 = pk[:,12]) ----
                hr678 = hrp.tile([128, MTILE], f16, tag="hr678")
                hm678 = hmp.tile([128, MTILE], f16, tag="hm678")

                z6 = psS.tile([128, MTILE], f32, tag="s")
                nc.tensor.matmul(z6[0:64, :], w6[:, 0, :], hm5[:], start=True, stop=True)
                nc.vector.tensor_scalar(hr678[0:64, :], z6[0:64, :], b678[0:64, 0:1], 0.0, ADD, MAX)
                nc.gpsimd.tensor_mul(hm678[0:64, :], hr678[0:64, :], pk[0:64, 12, :])

                z7 = psS.tile([128, MTILE], f32, tag="s")
                nc.tensor.matmul(z7[64:96, :], w789[0:64, 0:32], hm678[0:64, :],
                                 start=True, stop=True, tile_position=(0, 64))
                nc.vector.tensor_scalar(hr678[64:96, :], z7[64:96, :], b678[64:96, 0:1], 0.0, ADD, MAX)
                nc.gpsimd.tensor_mul(hm678[64:96, :], hr678[64:96, :], pk[64:96, 12, :])

                z8 = psS.tile([128, MTILE], f32, tag="s")
                nc.tensor.matmul(z8[96:112, :], w789[64:96, 32:48], hm678[64:96, :],
                                 start=True, stop=True, tile_position=(64, 96))
                nc.vector.tensor_scalar(hr678[96:112, :], z8[96:112, :], b678[96:112, 0:1], 0.0, ADD, MAX)
                nc.gpsimd.tensor_mul(hm678[96:112, :], hr678[96:112, :], pk[96:112, 12, :])

                # ---- L9: 16 -> 10, bias only ----
                z9 = psS.tile([128, MTILE], f32, tag="s")
                nc.tensor.matmul(z9[0:10, :], w789[96:112, 48:58], hm678[96:112, :],
                                 start=True, stop=True, tile_position=(96, 0))
                osb = outp.tile([10, MTILE], f32, tag="osb")
                nc.scalar.activation(osb[:], z9[0:10, :], IDENT, bias=b9[:, 0:1])
                nc.sync.dma_start(out_d[:, cs], osb[:])

    nc.compile()
    return nc


def _get_program():
    if "nc" not in _PROG:
        _PROG["nc"] = _build_program()
    return _PROG["nc"]


def _host_prep(inputs):
    """Build per-core input maps (numpy only)."""
    x = np.asarray(inputs["x"], dtype=np.float32)
    Ws = [np.asarray(inputs[f"W{i}"], dtype=np.float32) for i in range(1, 10)]
    bs = [np.asarray(inputs[f"b{i}"], dtype=np.float32) for i in range(1, 10)]
    ms = [np.asarray(inputs[f"m{i}"], dtype=np.float32) for i in range(1, 9)]

    # fold dropout scale into next layer's weights; binarize masks
    Wf = [Ws[0]]
    for i in range(1, 9):
        s = float(ms[i - 1].max())
        if s <= 0.0:  # degenerate all-dropped mask; keep weights unscaled
            s = 1.0
        Wf.append(Ws[i] * np.float32(s))

    def pack_w(W, parts):
        K, N = W.shape
        Kc = (K + 127) // 128
        out = np.zeros((parts, Kc, N), dtype=np.float16)
        for k in range(Kc):
            blk = W[k * 128:(k + 1) * 128]
            out[: blk.shape[0], k, :] = blk.astype(np.float16)
        return out

    shared = {
        "W1": pack_w(Wf[0], 128), "W2": pack_w(Wf[1], 128), "W3": pack_w(Wf[2], 128),
        "W4": pack_w(Wf[3], 128), "W5": pack_w(Wf[4], 128), "W6": pack_w(Wf[5], 128),
        "W7": Wf[6].astype(np.float16), "W8": Wf[7].astype(np.float16),
        "W9": Wf[8].astype(np.float16),
    }
    b15 = np.zeros((128, 10), dtype=np.float32)
    b15[:, 0] = bs[0]
    b15[:, 1], b15[:, 2] = bs[1][0:128], bs[1][128:256]
    for c in range(4):
        b15[:, 3 + c] = bs[2][c * 128:(c + 1) * 128]
    b15[:, 7], b15[:, 8] = bs[3][0:128], bs[3][128:256]
    b15[:, 9] = bs[4]
    b678 = np.zeros((128, 1), dtype=np.float32)
    b678[0:64, 0], b678[64:96, 0], b678[96:112, 0] = bs[5], bs[6], bs[7]
    shared["B15"], shared["B678"] = b15, b678
    shared["B9"] = bs[8].reshape(10, 1)

    in_maps = []
    for c in range(NCORES):
        sl = slice(c * SHARD, (c + 1) * SHARD)
        pack = np.zeros((128, NPACK, SHARD), dtype=np.float16)
        xT = x[sl].T  # (256, SHARD)
        pack[:, 0, :] = xT[0:128]
        pack[:, 1, :] = xT[128:256]
        mT = [None] + [(ms[i][sl] != 0).T.astype(np.float16) for i in range(8)]  # 1-indexed
        pack[:, 2, :] = mT[1]
        pack[:, 3, :], pack[:, 4, :] = mT[2][0:128], mT[2][128:256]
        for k in range(4):
            pack[:, 5 + k, :] = mT[3][k * 128:(k + 1) * 128]
        pack[:, 9, :], pack[:, 10, :] = mT[4][0:128], mT[4][128:256]
        pack[:, 11, :] = mT[5]
        pack[0:64, 12, :] = mT[6]
        pack[64:96, 12, :] = mT[7]
        pack[96:112, 12, :] = mT[8]
        in_maps.append({"pack": pack, **shared})
    return in_maps


def kernel(**inputs) -> np.ndarray:
    from concourse.bass_utils import run_bass_kernel_spmd

    nc = _get_program()
    in_maps = _host_prep(inputs)
    res = run_bass_kernel_spmd(nc, in_maps, list(range(NCORES)))
    out = np.empty((BATCH, DIMS[-1]), dtype=np.float32)
    for c in range(NCORES):
        out[c * SHARD:(c + 1) * SHARD, :] = res.results[c]["outT"].T
    return out


# revision 3
# speedup vs baseline: 1.0784x; 1.0784x over previous
"""Trainium2 Bass kernel for the 9-layer dense MLP (dropout-mask training forward).

Strategy (pure data parallel, 8 cores, 8192 batch rows each):
  - Activations kept transposed on-chip: features on partitions, batch cols on free dim.
    Each layer computes zT = W^T @ hT via nc.tensor.matmul(out, lhsT=W, rhs=hT).
  - fp16 weights/activations/masks (fp32 PSUM accumulation), fp32 biases + output.
  - Dropout masks binarized on host ({0,1} fp16); the 1/keep scale is folded into the
    next layer's weights.
  - Per batch tile of 512 columns: matmul chain L1..L9 with PSUM accumulation over
    K-chunks; PSUM drained with fused bias+relu (ScalarE activation or DVE
    tensor_scalar); mask multiply on DVE/GpSimd.
  - One packed-input DMA per tile (x chunks + all mask chunks interleaved host-side).
  - Layers 6/7/8 are partition-packed (outputs at partition offsets 0/64/96 via
    matmul tile_position) so their masks ride in one packed chunk.
"""

import sys

sys.path.insert(0, "/opt/trn_rl_repo")

import numpy as np

DIMS = [256, 128, 256, 512, 256, 128, 64, 32, 16, 10]
NCORES = 8
BATCH = 65536
SHARD = BATCH // NCORES  # 8192
MTILE = 512
NTILES = SHARD // MTILE  # 16

# pack chunk layout (each chunk = 128 partitions x 8192 cols, fp16):
#   0,1: xT        2: m1        3,4: m2      5-8: m3
#   9,10: m4       11: m5       12: m6/m7/m8 partition-packed at rows 0/64/96
NPACK = 13

_PROG = {}


def _build_program():
    import concourse.bass as bass
    import concourse.tile as tile
    from concourse import bacc, mybir

    f16 = mybir.dt.float16
    f32 = mybir.dt.float32
    RELU = mybir.ActivationFunctionType.Relu
    IDENT = mybir.ActivationFunctionType.Identity
    ADD = mybir.AluOpType.add
    MAX = mybir.AluOpType.max

    nc = bacc.Bacc("TRN2", target_bir_lowering=False, debug=False, num_devices=NCORES)

    pack_d = nc.dram_tensor("pack", [128, NPACK, SHARD], f16, kind="ExternalInput").ap()
    # lhsT-packed weights: (partitions, Kc, N) with K-chunk k at [:, k, :]
    w_d = {}
    w_d[1] = nc.dram_tensor("W1", [128, 2, 128], f16, kind="ExternalInput").ap()
    w_d[2] = nc.dram_tensor("W2", [128, 1, 256], f16, kind="ExternalInput").ap()
    w_d[3] = nc.dram_tensor("W3", [128, 2, 512], f16, kind="ExternalInput").ap()
    w_d[4] = nc.dram_tensor("W4", [128, 4, 256], f16, kind="ExternalInput").ap()
    w_d[5] = nc.dram_tensor("W5", [128, 2, 128], f16, kind="ExternalInput").ap()
    w_d[6] = nc.dram_tensor("W6", [128, 1, 64], f16, kind="ExternalInput").ap()
    w_d[7] = nc.dram_tensor("W7", [64, 32], f16, kind="ExternalInput").ap()
    w_d[8] = nc.dram_tensor("W8", [32, 16], f16, kind="ExternalInput").ap()
    w_d[9] = nc.dram_tensor("W9", [16, 10], f16, kind="ExternalInput").ap()
    b15_d = nc.dram_tensor("B15", [128, 10], f32, kind="ExternalInput").ap()
    b678_d = nc.dram_tensor("B678", [128, 1], f32, kind="ExternalInput").ap()
    b9_d = nc.dram_tensor("B9", [10, 1], f32, kind="ExternalInput").ap()
    out_d = nc.dram_tensor("outT", [10, SHARD], f32, kind="ExternalOutput").ap()

    with tile.TileContext(nc) as tc:
        with (
            tc.tile_pool(name="wpool", bufs=1) as wp,
            tc.tile_pool(name="pack", bufs=4) as packp,
            tc.tile_pool(name="hr", bufs=3) as hrp,
            tc.tile_pool(name="hm", bufs=3) as hmp,
            tc.tile_pool(name="osb", bufs=4) as outp,
            tc.tile_pool(name="ps", bufs=7, space="PSUM") as psp,
        ):
            w1 = wp.tile([128, 2, 128], f16, tag="w1")
            w2 = wp.tile([128, 1, 256], f16, tag="w2")
            w3 = wp.tile([128, 2, 512], f16, tag="w3")
            w4 = wp.tile([128, 4, 256], f16, tag="w4")
            w5 = wp.tile([128, 2, 128], f16, tag="w5")
            w6 = wp.tile([128, 1, 64], f16, tag="w6")
            w789 = wp.tile([128, 64], f16, tag="w789")  # W7 @[0:64,0:32], W8 @[64:96,32:48], W9 @[96:112,48:58]
            b15 = wp.tile([128, 10], f32, tag="b15")
            b678 = wp.tile([128, 1], f32, tag="b678")
            b9 = wp.tile([10, 1], f32, tag="b9")
            for t_, d_ in ((w1, w_d[1]), (w2, w_d[2]), (w3, w_d[3]), (w4, w_d[4]),
                           (w5, w_d[5]), (w6, w_d[6]), (b15, b15_d), (b678, b678_d), (b9, b9_d)):
                nc.sync.dma_start(t_[:], d_[:])
            nc.sync.dma_start(w789[0:64, 0:32], w_d[7][:])
            nc.sync.dma_start(w789[64:96, 32:48], w_d[8][:])
            nc.sync.dma_start(w789[96:112, 48:58], w_d[9][:])

            for t in range(NTILES):
                cs = bass.ts(t, MTILE)
                pk = packp.tile([128, NPACK, MTILE], f16, tag="pk")
                nc.sync.dma_start(pk[:], pack_d[:, :, cs])

                # ---- L1: 256 -> 128 (mask m1 = pk[:,2]) ----
                z1 = psp.tile([128, MTILE], f32, tag="ps")
                nc.tensor.matmul(z1[:], w1[:, 0, :], pk[:, 0, :], start=True, stop=False)
                nc.tensor.matmul(z1[:], w1[:, 1, :], pk[:, 1, :], start=False, stop=True)
                hr1 = hrp.tile([128, MTILE], f16, tag="hr1")
                nc.vector.tensor_scalar(hr1[:], z1[:], b15[:, 0:1], 0.0, ADD, MAX)
                hm1 = hmp.tile([128, MTILE], f16, tag="hm1")
                nc.gpsimd.tensor_mul(hm1[:], hr1[:], pk[:, 2, :])

                # ---- L2: 128 -> 256 (m2 = pk[:,3:5]) ----
                hr2 = hrp.tile([128, 2, MTILE], f16, tag="hr2")
                for c in range(2):
                    z2 = psp.tile([128, MTILE], f32, tag="ps")
                    nc.tensor.matmul(z2[:], w2[:, 0, bass.ts(c, 128)], hm1[:],
                                     start=True, stop=True)
                    nc.scalar.activation(hr2[:, c, :], z2[:], RELU, bias=b15[:, 1 + c:2 + c])
                hm2 = hmp.tile([128, 2, MTILE], f16, tag="hm2")
                nc.vector.tensor_mul(hm2[:], hr2[:], pk[:, 3:5, :])

                # ---- L3: 256 -> 512 (m3 = pk[:,5:9]) ----
                hr3 = hrp.tile([128, 4, MTILE], f16, tag="hr3")
                for c in range(4):
                    z3 = psp.tile([128, MTILE], f32, tag="ps")
                    for k in range(2):
                        nc.tensor.matmul(z3[:], w3[:, k, bass.ts(c, 128)],
                                         hm2[:, k, :], start=(k == 0), stop=(k == 1))
                    nc.scalar.activation(hr3[:, c, :], z3[:], RELU, bias=b15[:, 3 + c:4 + c])
                hm3 = hmp.tile([128, 4, MTILE], f16, tag="hm3")
                nc.vector.tensor_mul(hm3[:], hr3[:], pk[:, 5:9, :])

                # ---- L4: 512 -> 256 (m4 = pk[:,9:11]) ----
                hr4 = hrp.tile([128, 2, MTILE], f16, tag="hr4")
                for c in range(2):
                    z4 = psp.tile([128, MTILE], f32, tag="ps")
                    for k in range(4):
                        nc.tensor.matmul(z4[:], w4[:, k, bass.ts(c, 128)],
                                         hm3[:, k, :], start=(k == 0), stop=(k == 3))
                    nc.scalar.activation(hr4[:, c, :], z4[:], RELU, bias=b15[:, 7 + c:8 + c])
                hm4 = hmp.tile([128, 2, MTILE], f16, tag="hm4")
                nc.vector.tensor_mul(hm4[:], hr4[:], pk[:, 9:11, :])

                # ---- L5: 256 -> 128 (m5 = pk[:,11]) ----
                z5 = psp.tile([128, MTILE], f32, tag="ps")
                for k in range(2):
                    nc.tensor.matmul(z5[:], w5[:, k, :], hm4[:, k, :], start=(k == 0), stop=(k == 1))
                hr5 = hrp.tile([128, MTILE], f16, tag="hr5")
                nc.vector.tensor_scalar(hr5[:], z5[:], b15[:, 9:10], 0.0, ADD, MAX)
                hm5 = hmp.tile([128, MTILE], f16, tag="hm5")
                nc.gpsimd.tensor_mul(hm5[:], hr5[:], pk[:, 11, :])

                # ---- L6/L7/L8: partition-packed at 0/64/96 (m678 = pk[:,12]) ----
                hr678 = hrp.tile([128, MTILE], f16, tag="hr678")
                hm678 = hmp.tile([128, MTILE], f16, tag="hm678")

                z678 = psp.tile([128, MTILE], f32, tag="ps")
                nc.tensor.matmul(z678[0:64, :], w6[:, 0, :], hm5[:], start=True, stop=True)
                nc.vector.tensor_scalar(hr678[0:64, :], z678[0:64, :], b678[0:64, 0:1], 0.0, ADD, MAX)
                nc.gpsimd.tensor_mul(hm678[0:64, :], hr678[0:64, :], pk[0:64, 12, :])

                nc.tensor.matmul(z678[64:96, :], w789[0:64, 0:32], hm678[0:64, :],
                                 start=True, stop=True, tile_position=(0, 64))
                nc.vector.tensor_scalar(hr678[64:96, :], z678[64:96, :], b678[64:96, 0:1], 0.0, ADD, MAX)
                nc.gpsimd.tensor_mul(hm678[64:96, :], hr678[64:96, :], pk[64:96, 12, :])

                nc.tensor.matmul(z678[96:112, :], w789[64:96, 32:48], hm678[64:96, :],
                                 start=True, stop=True, tile_position=(64, 96))
                nc.vector.tensor_scalar(hr678[96:112, :], z678[96:112, :], b678[96:112, 0:1], 0.0, ADD, MAX)
                nc.gpsimd.tensor_mul(hm678[96:112, :], hr678[96:112, :], pk[96:112, 12, :])

                # ---- L9: 16 -> 10, bias only ----
                z9 = psp.tile([128, MTILE], f32, tag="ps")
                nc.tensor.matmul(z9[0:10, :], w789[96:112, 48:58], hm678[96:112, :],
                                 start=True, stop=True, tile_position=(96, 0))
                osb = outp.tile([10, MTILE], f32, tag="osb")
                nc.scalar.activation(osb[:], z9[0:10, :], IDENT, bias=b9[:, 0:1])
                nc.sync.dma_start(out_d[:, cs], osb[:])

    nc.compile()
    return nc


def _get_program():
    if "nc" not in _PROG:
        _PROG["nc"] = _build_program()
    return _PROG["nc"]


def _host_prep(inputs):
    """Build per-core input maps (numpy only)."""
    x = np.asarray(inputs["x"], dtype=np.float32)
    Ws = [np.asarray(inputs[f"W{i}"], dtype=np.float32) for i in range(1, 10)]
    bs = [np.asarray(inputs[f"b{i}"], dtype=np.float32) for i in range(1, 10)]
    ms = [np.asarray(inputs[f"m{i}"], dtype=np.float32) for i in range(1, 9)]

    # fold dropout scale into next layer's weights; binarize masks
    Wf = [Ws[0]]
    for i in range(1, 9):
        s = float(ms[i - 1].max())
        if s <= 0.0:  # degenerate all-dropped mask; keep weights unscaled
            s = 1.0
        Wf.append(Ws[i] * np.float32(s))

    def pack_w(W, parts):
        K, N = W.shape
        Kc = (K + 127) // 128
        out = np.zeros((parts, Kc, N), dtype=np.float16)
        for k in range(Kc):
            blk = W[k * 128:(k + 1) * 128]
            out[: blk.shape[0], k, :] = blk.astype(np.float16)
        return out

    shared = {
        "W1": pack_w(Wf[0], 128), "W2": pack_w(Wf[1], 128), "W3": pack_w(Wf[2], 128),
        "W4": pack_w(Wf[3], 128), "W5": pack_w(Wf[4], 128), "W6": pack_w(Wf[5], 128),
        "W7": Wf[6].astype(np.float16), "W8": Wf[7].astype(np.float16),
        "W9": Wf[8].astype(np.float16),
    }
    b15 = np.zeros((128, 10), dtype=np.float32)
    b15[:, 0] = bs[0]
    b15[:, 1], b15[:, 2] = bs[1][0:128], bs[1][128:256]
    for c in range(4):
        b15[:, 3 + c] = bs[2][c * 128:(c + 1) * 128]
    b15[:, 7], b15[:, 8] = bs[3][0:128], bs[3][128:256]
    b15[:, 9] = bs[4]
    b678 = np.zeros((128, 1), dtype=np.float32)
    b678[0:64, 0], b678[64:96, 0], b678[96:112, 0] = bs[5], bs[6], bs[7]
    shared["B15"], shared["B678"] = b15, b678
    shared["B9"] = bs[8].reshape(10, 1)

    in_maps = []
    for c in range(NCORES):
        sl = slice(c * SHARD, (c + 1) * SHARD)
        pack = np.zeros((128, NPACK, SHARD), dtype=np.float16)
        xT = x[sl].T  # (256, SHARD)
        pack[:, 0, :] = xT[0:128]
        pack[:, 1, :] = xT[128:256]
        mT = [None] + [(ms[i][sl] != 0).T.astype(np.float16) for i in range(8)]  # 1-indexed
        pack[:, 2, :] = mT[1]
        pack[:, 3, :], pack[:, 4, :] = mT[2][0:128], mT[2][128:256]
        for k in range(4):
            pack[:, 5 + k, :] = mT[3][k * 128:(k + 1) * 128]
        pack[:, 9, :], pack[:, 10, :] = mT[4][0:128], mT[4][128:256]
        pack[:, 11, :] = mT[5]
        pack[0:64, 12, :] = mT[6]
        pack[64:96, 12, :] = mT[7]
        pack[96:112, 12, :] = mT[8]
        in_maps.append({"pack": pack, **shared})
    return in_maps


def kernel(**inputs) -> np.ndarray:
    from concourse.bass_utils import run_bass_kernel_spmd

    nc = _get_program()
    in_maps = _host_prep(inputs)
    res = run_bass_kernel_spmd(nc, in_maps, list(range(NCORES)))
    out = np.empty((BATCH, DIMS[-1]), dtype=np.float32)
    for c in range(NCORES):
        out[c * SHARD:(c + 1) * SHARD, :] = res.results[c]["outT"].T
    return out


# revision 5
# speedup vs baseline: 1.3041x; 1.2093x over previous
"""Trainium2 Bass kernel for the 9-layer dense MLP (dropout-mask training forward).

Strategy (pure data parallel, 8 cores, 8192 batch rows each):
  - Activations kept transposed on-chip: features on partitions, batch cols on free dim.
    Each layer computes zT = W^T @ hT via nc.tensor.matmul(out, lhsT=W, rhs=hT).
  - fp16 weights/activations/masks (fp32 PSUM accumulation), fp32 biases + output.
  - Dropout masks binarized on host ({0,1} fp16); the 1/keep scale is folded into the
    next layer's weights.
  - Layer-major processing within blocks of 1024 batch columns (2 PSUM sub-tiles of
    512): all matmuls of a layer run as one dense PE burst, PSUM drained per 512-chunk
    with fused bias+relu (ScalarE activation or DVE tensor_scalar), mask multiply
    batched per layer on DVE/GpSimd.
  - Layers 6/7/8 are partition-packed (outputs at partition offsets 0/64/96 via
    matmul tile_position) sharing one PSUM tile and one packed mask chunk.
"""

import sys

sys.path.insert(0, "/opt/trn_rl_repo")

import numpy as np

DIMS = [256, 128, 256, 512, 256, 128, 64, 32, 16, 10]
NCORES = 8
BATCH = 65536
SHARD = BATCH // NCORES  # 8192
MSUB = 512               # PSUM sub-tile columns
BLK = 1024               # block columns (2 sub-tiles)
NBLK = SHARD // BLK      # 8
NSUB = BLK // MSUB       # 2

# pack chunk layout (each chunk = 128 partitions x 8192 cols, fp16):
#   0,1: xT        2: m1        3,4: m2      5-8: m3
#   9,10: m4       11: m5       12: m6/m7/m8 partition-packed at rows 0/64/96
NPACK = 13

_PROG = {}


def _build_program():
    import concourse.bass as bass
    import concourse.tile as tile
    from concourse import bacc, mybir

    f16 = mybir.dt.float16
    f32 = mybir.dt.float32
    RELU = mybir.ActivationFunctionType.Relu
    IDENT = mybir.ActivationFunctionType.Identity
    ADD = mybir.AluOpType.add
    MAX = mybir.AluOpType.max

    nc = bacc.Bacc("TRN2", target_bir_lowering=False, debug=False, num_devices=NCORES)

    pack_d = nc.dram_tensor("pack", [128, NPACK, SHARD], f16, kind="ExternalInput").ap()
    w_d = {}
    w_d[1] = nc.dram_tensor("W1", [128, 2, 128], f16, kind="ExternalInput").ap()
    w_d[2] = nc.dram_tensor("W2", [128, 1, 256], f16, kind="ExternalInput").ap()
    w_d[3] = nc.dram_tensor("W3", [128, 2, 512], f16, kind="ExternalInput").ap()
    w_d[4] = nc.dram_tensor("W4", [128, 4, 256], f16, kind="ExternalInput").ap()
    w_d[5] = nc.dram_tensor("W5", [128, 2, 128], f16, kind="ExternalInput").ap()
    w_d[6] = nc.dram_tensor("W6", [128, 1, 64], f16, kind="ExternalInput").ap()
    w_d[7] = nc.dram_tensor("W7", [64, 32], f16, kind="ExternalInput").ap()
    w_d[8] = nc.dram_tensor("W8", [32, 16], f16, kind="ExternalInput").ap()
    w_d[9] = nc.dram_tensor("W9", [16, 10], f16, kind="ExternalInput").ap()
    b15_d = nc.dram_tensor("B15", [128, 10], f32, kind="ExternalInput").ap()
    b678_d = nc.dram_tensor("B678", [128, 1], f32, kind="ExternalInput").ap()
    b9_d = nc.dram_tensor("B9", [10, 1], f32, kind="ExternalInput").ap()
    out_d = nc.dram_tensor("outT", [10, SHARD], f32, kind="ExternalOutput").ap()

    with tile.TileContext(nc) as tc:
        with (
            tc.tile_pool(name="wpool", bufs=1) as wp,
            tc.tile_pool(name="mk", bufs=3) as mkp,
            tc.tile_pool(name="hr", bufs=2) as hrp,
            tc.tile_pool(name="hm", bufs=2) as hmp,
            tc.tile_pool(name="osb", bufs=3) as outp,
            tc.tile_pool(name="ps", bufs=6, space="PSUM") as psp,
            tc.tile_pool(name="ps678", bufs=1, space="PSUM") as ps678p,
        ):
            w1 = wp.tile([128, 2, 128], f16, tag="w1")
            w2 = wp.tile([128, 1, 256], f16, tag="w2")
            w3 = wp.tile([128, 2, 512], f16, tag="w3")
            w4 = wp.tile([128, 4, 256], f16, tag="w4")
            w5 = wp.tile([128, 2, 128], f16, tag="w5")
            w6 = wp.tile([128, 1, 64], f16, tag="w6")
            w789 = wp.tile([128, 64], f16, tag="w789")
            b15 = wp.tile([128, 10], f32, tag="b15")
            b678 = wp.tile([128, 1], f32, tag="b678")
            b9 = wp.tile([10, 1], f32, tag="b9")
            for t_, d_ in ((w1, w_d[1]), (w2, w_d[2]), (w3, w_d[3]), (w4, w_d[4]),
                           (w5, w_d[5]), (w6, w_d[6]), (b15, b15_d), (b678, b678_d), (b9, b9_d)):
                nc.sync.dma_start(t_[:], d_[:])
            nc.sync.dma_start(w789[0:64, 0:32], w_d[7][:])
            nc.sync.dma_start(w789[64:96, 32:48], w_d[8][:])
            nc.sync.dma_start(w789[96:112, 48:58], w_d[9][:])

            for b in range(NBLK):
                bs = bass.ts(b, BLK)
                pkx = mkp.tile([128, 2, BLK], f16, tag="pkx")
                m1 = mkp.tile([128, 1, BLK], f16, tag="m1")
                m2 = mkp.tile([128, 2, BLK], f16, tag="m2")
                m3 = mkp.tile([128, 4, BLK], f16, tag="m3")
                m4 = mkp.tile([128, 2, BLK], f16, tag="m4")
                m5 = mkp.tile([128, 1, BLK], f16, tag="m5")
                m678 = mkp.tile([128, 1, BLK], f16, tag="m678")
                nc.sync.dma_start(pkx[:], pack_d[:, 0:2, bs])
                nc.sync.dma_start(m1[:], pack_d[:, 2:3, bs])
                nc.sync.dma_start(m2[:], pack_d[:, 3:5, bs])
                nc.sync.dma_start(m3[:], pack_d[:, 5:9, bs])
                nc.sync.dma_start(m4[:], pack_d[:, 9:11, bs])
                nc.sync.dma_start(m5[:], pack_d[:, 11:12, bs])
                nc.sync.dma_start(m678[:], pack_d[:, 12:13, bs])

                def sub(ap3, c, t):
                    # (128, C, BLK) tile -> (parts, 512) slice of chunk c, sub-tile t
                    return ap3[:, c, bass.ts(t, MSUB)]

                # ---- L1: 256 -> 128 ----
                z1 = [psp.tile([128, MSUB], f32, tag="ps", name=f"z1_{b}_{i}") for i in range(NSUB)]
                for t in range(NSUB):
                    for k in range(2):
                        nc.tensor.matmul(z1[t][:], w1[:, k, :], sub(pkx, k, t),
                                         start=(k == 0), stop=(k == 1))
                hr1 = hrp.tile([128, 1, BLK], f16, tag="hr1")
                for t in range(NSUB):
                    nc.vector.tensor_scalar(sub(hr1, 0, t), z1[t][:], b15[:, 0:1], 0.0, ADD, MAX)
                hm1 = hmp.tile([128, 1, BLK], f16, tag="hm1")
                nc.gpsimd.tensor_mul(hm1[:], hr1[:], m1[:])

                # ---- L2: 128 -> 256 ----
                hr2 = hrp.tile([128, 2, BLK], f16, tag="hr2")
                for c in range(2):
                    for t in range(NSUB):
                        z2 = psp.tile([128, MSUB], f32, tag="ps")
                        nc.tensor.matmul(z2[:], w2[:, 0, bass.ts(c, 128)], sub(hm1, 0, t),
                                         start=True, stop=True)
                        nc.scalar.activation(sub(hr2, c, t), z2[:], RELU, bias=b15[:, 1 + c:2 + c])
                hm2 = hmp.tile([128, 2, BLK], f16, tag="hm2")
                nc.vector.tensor_mul(hm2[:], hr2[:], m2[:])

                # ---- L3: 256 -> 512 ----
                hr3 = hrp.tile([128, 4, BLK], f16, tag="hr3")
                for c in range(4):
                    for t in range(NSUB):
                        z3 = psp.tile([128, MSUB], f32, tag="ps")
                        for k in range(2):
                            nc.tensor.matmul(z3[:], w3[:, k, bass.ts(c, 128)], sub(hm2, k, t),
                                             start=(k == 0), stop=(k == 1))
                        nc.scalar.activation(sub(hr3, c, t), z3[:], RELU, bias=b15[:, 3 + c:4 + c])
                hm3 = hmp.tile([128, 4, BLK], f16, tag="hm3")
                nc.vector.tensor_mul(hm3[:], hr3[:], m3[:])

                # ---- L4: 512 -> 256 ----
                hr4 = hrp.tile([128, 2, BLK], f16, tag="hr4")
                for c in range(2):
                    for t in range(NSUB):
                        z4 = psp.tile([128, MSUB], f32, tag="ps")
                        for k in range(4):
                            nc.tensor.matmul(z4[:], w4[:, k, bass.ts(c, 128)], sub(hm3, k, t),
                                             start=(k == 0), stop=(k == 3))
                        nc.scalar.activation(sub(hr4, c, t), z4[:], RELU, bias=b15[:, 7 + c:8 + c])
                hm4 = hmp.tile([128, 2, BLK], f16, tag="hm4")
                nc.vector.tensor_mul(hm4[:], hr4[:], m4[:])

                # ---- L5: 256 -> 128 ----
                z5 = [psp.tile([128, MSUB], f32, tag="ps", name=f"z5_{b}_{i}") for i in range(NSUB)]
                for t in range(NSUB):
                    for k in range(2):
                        nc.tensor.matmul(z5[t][:], w5[:, k, :], sub(hm4, k, t),
                                         start=(k == 0), stop=(k == 1))
                hr5 = hrp.tile([128, 1, BLK], f16, tag="hr5")
                for t in range(NSUB):
                    nc.vector.tensor_scalar(sub(hr5, 0, t), z5[t][:], b15[:, 9:10], 0.0, ADD, MAX)
                hm5 = hmp.tile([128, 1, BLK], f16, tag="hm5")
                nc.gpsimd.tensor_mul(hm5[:], hr5[:], m5[:])

                # ---- L6/L7/L8 partition-packed at 0/64/96, shared 2-bank PSUM ----
                z678 = ps678p.tile([128, BLK], f32, tag="z678")
                hr678 = hrp.tile([128, 1, BLK], f16, tag="hr678")
                hm678 = hmp.tile([128, 1, BLK], f16, tag="hm678")

                for t in range(NSUB):
                    nc.tensor.matmul(z678[0:64, bass.ts(t, MSUB)], w6[:, 0, :], sub(hm5, 0, t),
                                     start=True, stop=True)
                nc.vector.tensor_scalar(hr678[0:64, 0, :], z678[0:64, :], b678[0:64, 0:1], 0.0, ADD, MAX)
                nc.gpsimd.tensor_mul(hm678[0:64, 0, :], hr678[0:64, 0, :], m678[0:64, 0, :])

                for t in range(NSUB):
                    nc.tensor.matmul(z678[64:96, bass.ts(t, MSUB)], w789[0:64, 0:32],
                                     hm678[0:64, 0, bass.ts(t, MSUB)],
                                     start=True, stop=True, tile_position=(0, 64))
                nc.vector.tensor_scalar(hr678[64:96, 0, :], z678[64:96, :], b678[64:96, 0:1], 0.0, ADD, MAX)
                nc.gpsimd.tensor_mul(hm678[64:96, 0, :], hr678[64:96, 0, :], m678[64:96, 0, :])

                for t in range(NSUB):
                    nc.tensor.matmul(z678[96:112, bass.ts(t, MSUB)], w789[64:96, 32:48],
                                     hm678[64:96, 0, bass.ts(t, MSUB)],
                                     start=True, stop=True, tile_position=(64, 96))
                nc.vector.tensor_scalar(hr678[96:112, 0, :], z678[96:112, :], b678[96:112, 0:1], 0.0, ADD, MAX)
                nc.gpsimd.tensor_mul(hm678[96:112, 0, :], hr678[96:112, 0, :], m678[96:112, 0, :])

                # ---- L9: 16 -> 10, bias only ----
                osb = outp.tile([10, BLK], f32, tag="osb")
                for t in range(NSUB):
                    z9 = psp.tile([128, MSUB], f32, tag="ps")
                    nc.tensor.matmul(z9[0:10, :], w789[96:112, 48:58],
                                     hm678[96:112, 0, bass.ts(t, MSUB)],
                                     start=True, stop=True, tile_position=(96, 0))
                    nc.scalar.activation(osb[:, bass.ts(t, MSUB)], z9[0:10, :], IDENT, bias=b9[:, 0:1])
                nc.sync.dma_start(out_d[:, bs], osb[:])

    nc.compile()
    return nc


def _get_program():
    if "nc" not in _PROG:
        _PROG["nc"] = _build_program()
    return _PROG["nc"]


def _host_prep(inputs):
    """Build per-core input maps (numpy only)."""
    x = np.asarray(inputs["x"], dtype=np.float32)
    Ws = [np.asarray(inputs[f"W{i}"], dtype=np.float32) for i in range(1, 10)]
    bs = [np.asarray(inputs[f"b{i}"], dtype=np.float32) for i in range(1, 10)]
    ms = [np.asarray(inputs[f"m{i}"], dtype=np.float32) for i in range(1, 9)]

    # fold dropout scale into next layer's weights; binarize masks
    Wf = [Ws[0]]
    for i in range(1, 9):
        s = float(ms[i - 1].max())
        if s <= 0.0:  # degenerate all-dropped mask; keep weights unscaled
            s = 1.0
        Wf.append(Ws[i] * np.float32(s))

    def pack_w(W, parts):
        K, N = W.shape
        Kc = (K + 127) // 128
        out = np.zeros((parts, Kc, N), dtype=np.float16)
        for k in range(Kc):
            blk = W[k * 128:(k + 1) * 128]
            out[: blk.shape[0], k, :] = blk.astype(np.float16)
        return out

    shared = {
        "W1": pack_w(Wf[0], 128), "W2": pack_w(Wf[1], 128), "W3": pack_w(Wf[2], 128),
        "W4": pack_w(Wf[3], 128), "W5": pack_w(Wf[4], 128), "W6": pack_w(Wf[5], 128),
        "W7": Wf[6].astype(np.float16), "W8": Wf[7].astype(np.float16),
        "W9": Wf[8].astype(np.float16),
    }
    b15 = np.zeros((128, 10), dtype=np.float32)
    b15[:, 0] = bs[0]
    b15[:, 1], b15[:, 2] = bs[1][0:128], bs[1][128:256]
    for c in range(4):
        b15[:, 3 + c] = bs[2][c * 128:(c + 1) * 128]
    b15[:, 7], b15[:, 8] = bs[3][0:128], bs[3][128:256]
    b15[:, 9] = bs[4]
    b678 = np.zeros((128, 1), dtype=np.float32)
    b678[0:64, 0], b678[64:96, 0], b678[96:112, 0] = bs[5], bs[6], bs[7]
    shared["B15"], shared["B678"] = b15, b678
    shared["B9"] = bs[8].reshape(10, 1)

    in_maps = []
    for c in range(NCORES):
        sl = slice(c * SHARD, (c + 1) * SHARD)
        pack = np.zeros((128, NPACK, SHARD), dtype=np.float16)
        xT = x[sl].T  # (256, SHARD)
        pack[:, 0, :] = xT[0:128]
        pack[:, 1, :] = xT[128:256]
        mT = [None] + [(ms[i][sl] != 0).T.astype(np.float16) for i in range(8)]  # 1-indexed
        pack[:, 2, :] = mT[1]
        pack[:, 3, :], pack[:, 4, :] = mT[2][0:128], mT[2][128:256]
        for k in range(4):
            pack[:, 5 + k, :] = mT[3][k * 128:(k + 1) * 128]
        pack[:, 9, :], pack[:, 10, :] = mT[4][0:128], mT[4][128:256]
        pack[:, 11, :] = mT[5]
        pack[0:64, 12, :] = mT[6]
        pack[64:96, 12, :] = mT[7]
        pack[96:112, 12, :] = mT[8]
        in_maps.append({"pack": pack, **shared})
    return in_maps


def kernel(**inputs) -> np.ndarray:
    from concourse.bass_utils import run_bass_kernel_spmd

    nc = _get_program()
    in_maps = _host_prep(inputs)
    res = run_bass_kernel_spmd(nc, in_maps, list(range(NCORES)))
    out = np.empty((BATCH, DIMS[-1]), dtype=np.float32)
    for c in range(NCORES):
        out[c * SHARD:(c + 1) * SHARD, :] = res.results[c]["outT"].T
    return out


# revision 11
# speedup vs baseline: 2.1039x; 1.6133x over previous
"""Trainium2 Bass kernel for the 9-layer dense MLP (dropout-mask training forward).

Strategy (pure data parallel, 8 cores, 8192 batch rows each):
  - Activations kept transposed on-chip: features on partitions, batch cols on free dim.
    Each layer computes zT = W^T @ hT via nc.tensor.matmul(out, lhsT=W, rhs=hT).
  - fp16 weights/activations/masks (fp32 PSUM accumulation), fp32 biases + output.
  - Dropout masks binarized on host ({0,1} fp16); the 1/keep scale is folded into the
    next layer's weights.
  - Layer-major processing within blocks of 2048 batch columns (4 PSUM sub-tiles of
    512): per (c,k) weight tile, 4 consecutive matmuls share one LDWEIGHTS and
    pipeline back-to-back on the PE; PSUM drained per 512-chunk with fused bias+relu
    alternating ScalarE/VectorE; mask multiply per (layer, sub-tile) on DVE/GpSimd.
  - Layers 6/7/8 partition-packed (partition offsets 0/64/96 via matmul tile_position)
    sharing 2-bank PSUM tiles per 1024-column half and one packed mask chunk.
"""

import sys

sys.path.insert(0, "/opt/trn_rl_repo")

import numpy as np

DIMS = [256, 128, 256, 512, 256, 128, 64, 32, 16, 10]
NCORES = 8
BATCH = 65536
SHARD = BATCH // NCORES  # 8192
MSUB = 512               # PSUM sub-tile columns
BLK = 2048               # block columns
NBLK = SHARD // BLK      # 4
NSUB = BLK // MSUB       # 4

# pack chunk layout (each chunk = 128 partitions x 8192 cols, fp16):
#   0,1: xT        2: m1        3,4: m2      5-8: m3
#   9,10: m4       11: m5       12: m6/m7/m8 partition-packed at rows 0/64/96
NPACK = 13

_PROG = {}


def _raise_sbuf_cap():
    # tile_utils.max_sbuf_usage is a stale 192KB constant; cayman has 208KB usable.
    import concourse.tile_utils as tu

    if getattr(tu, "max_sbuf_usage", 0) < 206 * 1024:
        tu.max_sbuf_usage = 206 * 1024


def _build_program():
    import concourse.bass as bass
    import concourse.tile as tile
    from concourse import bacc, mybir

    _raise_sbuf_cap()

    f16 = mybir.dt.float16
    f32 = mybir.dt.float32
    RELU = mybir.ActivationFunctionType.Relu
    IDENT = mybir.ActivationFunctionType.Identity
    ADD = mybir.AluOpType.add
    MAX = mybir.AluOpType.max

    nc = bacc.Bacc("TRN2", target_bir_lowering=False, debug=False, num_devices=NCORES)

    pack_d = nc.dram_tensor("pack", [128, NPACK, SHARD], f16, kind="ExternalInput").ap()
    w_d = {}
    w_d[1] = nc.dram_tensor("W1", [128, 2, 128], f16, kind="ExternalInput").ap()
    w_d[2] = nc.dram_tensor("W2", [128, 1, 256], f16, kind="ExternalInput").ap()
    w_d[3] = nc.dram_tensor("W3", [128, 2, 512], f16, kind="ExternalInput").ap()
    w_d[4] = nc.dram_tensor("W4", [128, 4, 256], f16, kind="ExternalInput").ap()
    w_d[5] = nc.dram_tensor("W5", [128, 2, 128], f16, kind="ExternalInput").ap()
    w_d[6] = nc.dram_tensor("W6", [128, 1, 64], f16, kind="ExternalInput").ap()
    w_d[7] = nc.dram_tensor("W7", [64, 32], f16, kind="ExternalInput").ap()
    w_d[8] = nc.dram_tensor("W8", [32, 16], f16, kind="ExternalInput").ap()
    w_d[9] = nc.dram_tensor("W9", [16, 10], f16, kind="ExternalInput").ap()
    b15_d = nc.dram_tensor("B15", [128, 10], f32, kind="ExternalInput").ap()
    b678_d = nc.dram_tensor("B678", [128, 1], f32, kind="ExternalInput").ap()
    b9_d = nc.dram_tensor("B9", [10, 1], f32, kind="ExternalInput").ap()
    out_d = nc.dram_tensor("outT", [10, SHARD], f32, kind="ExternalOutput").ap()

    with tile.TileContext(nc) as tc:
        with (
            tc.tile_pool(name="wpool", bufs=1) as wp,
            tc.tile_pool(name="mk", bufs=2) as mkp,
            tc.tile_pool(name="hr", bufs=1) as hrp,
            tc.tile_pool(name="hm", bufs=1) as hmp,
            tc.tile_pool(name="osb", bufs=2) as outp,
            tc.tile_pool(name="ps", bufs=6, space="PSUM") as psp,
            tc.tile_pool(name="ps678", bufs=1, space="PSUM") as ps678p,
        ):
            w1 = wp.tile([128, 2, 128], f16, tag="w1")
            w2 = wp.tile([128, 1, 256], f16, tag="w2")
            w3 = wp.tile([128, 2, 512], f16, tag="w3")
            w4 = wp.tile([128, 4, 256], f16, tag="w4")
            w5 = wp.tile([128, 2, 128], f16, tag="w5")
            w6 = wp.tile([128, 1, 64], f16, tag="w6")
            w789 = wp.tile([128, 64], f16, tag="w789")
            b15 = wp.tile([128, 10], f32, tag="b15")
            b678 = wp.tile([128, 1], f32, tag="b678")
            b9 = wp.tile([10, 1], f32, tag="b9")
            for t_, d_ in ((w1, w_d[1]), (w2, w_d[2]), (w3, w_d[3]), (w4, w_d[4]),
                           (w5, w_d[5]), (w6, w_d[6]), (b15, b15_d), (b678, b678_d), (b9, b9_d)):
                nc.sync.dma_start(t_[:], d_[:])
            nc.sync.dma_start(w789[0:64, 0:32], w_d[7][:])
            nc.sync.dma_start(w789[64:96, 32:48], w_d[8][:])
            nc.sync.dma_start(w789[96:112, 48:58], w_d[9][:])

            for b in range(NBLK):
                bs = bass.ts(b, BLK)
                pkx = mkp.tile([128, 2, BLK], f16, tag="pkx")
                m1 = mkp.tile([128, 1, BLK], f16, tag="m1")
                m2 = mkp.tile([128, 2, BLK], f16, tag="m2")
                m3 = mkp.tile([128, 4, BLK], f16, tag="m3", bufs=1)
                m4 = mkp.tile([128, 2, BLK], f16, tag="m4")
                m5 = mkp.tile([128, 1, BLK], f16, tag="m5")
                m678 = mkp.tile([128, 1, BLK], f16, tag="m678")
                nc.sync.dma_start(pkx[:], pack_d[:, 0:2, bs])
                nc.sync.dma_start(m1[:], pack_d[:, 2:3, bs])
                nc.sync.dma_start(m2[:], pack_d[:, 3:5, bs])
                nc.sync.dma_start(m3[:], pack_d[:, 5:9, bs])
                nc.sync.dma_start(m4[:], pack_d[:, 9:11, bs])
                nc.sync.dma_start(m5[:], pack_d[:, 11:12, bs])
                nc.sync.dma_start(m678[:], pack_d[:, 12:13, bs])

                def sub(ap3, c, t):
                    return ap3[:, c, bass.ts(t, MSUB)]

                def drain_relu(eng, dst, zsrc, bias_ap):
                    if eng == "act":
                        nc.scalar.activation(dst, zsrc, RELU, bias=bias_ap)
                    else:
                        nc.vector.tensor_scalar(dst, zsrc, bias_ap, 0.0, ADD, MAX)

                def mask_mul(eng, dst, src, msrc):
                    if eng == "gps":
                        nc.gpsimd.tensor_mul(dst, src, msrc)
                    else:
                        nc.vector.tensor_mul(dst, src, msrc)

                dr_i = [0]

                def pick_drain():
                    i = dr_i[0]
                    dr_i[0] += 1
                    return "act" if (i * 3) % 5 < 3 else "dve"

                # big layers, weight-major inner order: per (c,k) one LDW, NSUB
                # back-to-back matmuls
                hin = pkx
                layer_cfg = [
                    (2, w1, 1, m1, 0, "hr1", "hm1"),
                    (1, w2, 2, m2, 1, "hr2", "hm2"),
                    (2, w3, 4, m3, 3, "hr3", "hm3"),
                    (4, w4, 2, m4, 7, "hr4", "hm4"),
                    (2, w5, 1, m5, 9, "hr5", "hm5"),
                ]
                for (Kc, wt, Cc, mt, boff, hrtag, hmtag) in layer_cfg:
                    hr = hrp.tile([128, Cc, BLK], f16, tag=hrtag, name=hrtag + f"_{b}")
                    hm = hmp.tile([128, Cc, BLK], f16, tag=hmtag, name=hmtag + f"_{b}")
                    zs = {}
                    for c in range(Cc):
                        for t in range(NSUB):
                            zs[c, t] = psp.tile([128, MSUB], f32, tag="ps",
                                                name=f"z_{hrtag}_{b}_{c}_{t}")
                    for c in range(Cc):
                        for k in range(Kc):
                            wap = wt[:, k, bass.ts(c, 128)] if Cc > 1 else wt[:, k, :]
                            for t in range(NSUB):
                                nc.tensor.matmul(zs[c, t][:], wap, sub(hin, k, t),
                                                 start=(k == 0), stop=(k == Kc - 1))
                    for t in range(NSUB):
                        for c in range(Cc):
                            drain_relu(pick_drain(), sub(hr, c, t), zs[c, t][:],
                                       b15[:, boff + c:boff + c + 1])
                        meng = "gps" if (Cc == 1 and t % 2 == 1) else "dve"
                        mask_mul(meng, hr[:, :, bass.ts(t, MSUB)] if False else hm[:, :, bass.ts(t, MSUB)],
                                 hr[:, :, bass.ts(t, MSUB)], mt[:, :, bass.ts(t, MSUB)])
                    hin = hm

                hm5 = hin

                # ---- L6/L7/L8 partition-packed; PSUM per 1024-col half ----
                hr678 = hrp.tile([128, 1, BLK], f16, tag="hr678")
                hm678 = hmp.tile([128, 1, BLK], f16, tag="hm678")
                zh = [ps678p.tile([128, 2 * MSUB], f32, tag="z678", name=f"z678_{b}_{h}")
                      for h in range(NSUB // 2)]

                def ladder(prange, wap, tile_pos, brange):
                    p0, p1 = prange
                    for t in range(NSUB):
                        half, off = zh[t // 2], (t % 2) * MSUB
                        ts_ = bass.ts(t, MSUB)
                        rhs = (sub(hm5, 0, t) if p0 == 0 else
                               hm678[brange[0]:brange[1], 0, ts_])
                        if tile_pos is None:
                            nc.tensor.matmul(half[p0:p1, off:off + MSUB], wap, rhs,
                                             start=True, stop=True)
                        else:
                            nc.tensor.matmul(half[p0:p1, off:off + MSUB], wap, rhs,
                                             start=True, stop=True, tile_position=tile_pos)
                    for t in range(NSUB):
                        half, off = zh[t // 2], (t % 2) * MSUB
                        ts_ = bass.ts(t, MSUB)
                        drain_relu("dve" if t % 2 == 0 else "act",
                                   hr678[p0:p1, 0, ts_], half[p0:p1, off:off + MSUB],
                                   b678[p0:p1, 0:1])
                        mask_mul("dve" if t % 2 == 0 else "gps",
                                 hm678[p0:p1, 0, ts_], hr678[p0:p1, 0, ts_],
                                 m678[p0:p1, 0, ts_])

                ladder((0, 64), w6[:, 0, :], None, None)
                ladder((64, 96), w789[0:64, 0:32], (0, 64), (0, 64))
                ladder((96, 112), w789[64:96, 32:48], (64, 96), (64, 96))

                # ---- L9: 16 -> 10, bias only ----
                osb = outp.tile([10, BLK], f32, tag="osb", bufs=1)
                for t in range(NSUB):
                    z9 = psp.tile([128, MSUB], f32, tag="ps", name=f"z9_{b}_{t}")
                    nc.tensor.matmul(z9[0:10, :], w789[96:112, 48:58],
                                     hm678[96:112, 0, bass.ts(t, MSUB)],
                                     start=True, stop=True, tile_position=(96, 0))
                    nc.scalar.activation(osb[:, bass.ts(t, MSUB)], z9[0:10, :], IDENT, bias=b9[:, 0:1])
                nc.sync.dma_start(out_d[:, bs], osb[:])

    nc.compile()
    return nc


def _get_program():
    if "nc" not in _PROG:
        _PROG["nc"] = _build_program()
    return _PROG["nc"]


def _host_prep(inputs):
    """Build per-core input maps (numpy only)."""
    x = np.asarray(inputs["x"], dtype=np.float32)
    Ws = [np.asarray(inputs[f"W{i}"], dtype=np.float32) for i in range(1, 10)]
    bs = [np.asarray(inputs[f"b{i}"], dtype=np.float32) for i in range(1, 10)]
    ms = [np.asarray(inputs[f"m{i}"], dtype=np.float32) for i in range(1, 9)]

    # fold dropout scale into next layer's weights; binarize masks
    Wf = [Ws[0]]
    for i in range(1, 9):
        s = float(ms[i - 1].max())
        if s <= 0.0:  # degenerate all-dropped mask; keep weights unscaled
            s = 1.0
        Wf.append(Ws[i] * np.float32(s))

    def pack_w(W, parts):
        K, N = W.shape
        Kc = (K + 127) // 128
        out = np.zeros((parts, Kc, N), dtype=np.float16)
        for k in range(Kc):
            blk = W[k * 128:(k + 1) * 128]
            out[: blk.shape[0], k, :] = blk.astype(np.float16)
        return out

    shared = {
        "W1": pack_w(Wf[0], 128), "W2": pack_w(Wf[1], 128), "W3": pack_w(Wf[2], 128),
        "W4": pack_w(Wf[3], 128), "W5": pack_w(Wf[4], 128), "W6": pack_w(Wf[5], 128),
        "W7": Wf[6].astype(np.float16), "W8": Wf[7].astype(np.float16),
        "W9": Wf[8].astype(np.float16),
    }
    b15 = np.zeros((128, 10), dtype=np.float32)
    b15[:, 0] = bs[0]
    b15[:, 1], b15[:, 2] = bs[1][0:128], bs[1][128:256]
    for c in range(4):
        b15[:, 3 + c] = bs[2][c * 128:(c + 1) * 128]
    b15[:, 7], b15[:, 8] = bs[3][0:128], bs[3][128:256]
    b15[:, 9] = bs[4]
    b678 = np.zeros((128, 1), dtype=np.float32)
    b678[0:64, 0], b678[64:96, 0], b678[96:112, 0] = bs[5], bs[6], bs[7]
    shared["B15"], shared["B678"] = b15, b678
    shared["B9"] = bs[8].reshape(10, 1)

    in_maps = []
    for c in range(NCORES):
        sl = slice(c * SHARD, (c + 1) * SHARD)
        pack = np.zeros((128, NPACK, SHARD), dtype=np.float16)
        xT = x[sl].T  # (256, SHARD)
        pack[:, 0, :] = xT[0:128]
        pack[:, 1, :] = xT[128:256]
        mT = [None] + [(ms[i][sl] != 0).T.astype(np.float16) for i in range(8)]  # 1-indexed
        pack[:, 2, :] = mT[1]
        pack[:, 3, :], pack[:, 4, :] = mT[2][0:128], mT[2][128:256]
        for k in range(4):
            pack[:, 5 + k, :] = mT[3][k * 128:(k + 1) * 128]
        pack[:, 9, :], pack[:, 10, :] = mT[4][0:128], mT[4][128:256]
        pack[:, 11, :] = mT[5]
        pack[0:64, 12, :] = mT[6]
        pack[64:96, 12, :] = mT[7]
        pack[96:112, 12, :] = mT[8]
        in_maps.append({"pack": pack, **shared})
    return in_maps


def kernel(**inputs) -> np.ndarray:
    from concourse.bass_utils import run_bass_kernel_spmd

    nc = _get_program()
    in_maps = _host_prep(inputs)
    res = run_bass_kernel_spmd(nc, in_maps, list(range(NCORES)))
    out = np.empty((BATCH, DIMS[-1]), dtype=np.float32)
    for c in range(NCORES):
        out[c * SHARD:(c + 1) * SHARD, :] = res.results[c]["outT"].T
    return out


# revision 13
# speedup vs baseline: 2.1055x; 1.0008x over previous
"""Trainium2 Bass kernel for the 9-layer dense MLP (dropout-mask training forward).

Strategy (pure data parallel, 8 cores, 8192 batch rows each):
  - Activations kept transposed on-chip: features on partitions, batch cols on free dim.
    Each layer computes zT = W^T @ hT via nc.tensor.matmul(out, lhsT=W, rhs=hT).
  - fp16 weights/activations/masks (fp32 PSUM accumulation), fp32 biases + output.
  - Dropout masks binarized on host ({0,1} fp16); the 1/keep scale is folded into the
    next layer's weights.
  - Layer-major processing within blocks of 2048 batch columns (4 PSUM sub-tiles of
    512): per (c,k) weight tile, 4 consecutive matmuls share one LDWEIGHTS and
    pipeline back-to-back on the PE; PSUM drained per 512-chunk with fused bias+relu
    alternating ScalarE/VectorE; mask multiply per (layer, sub-tile) on DVE/GpSimd.
  - Layers 6/7/8 partition-packed (partition offsets 0/64/96 via matmul tile_position)
    sharing 2-bank PSUM tiles per 1024-column half and one packed mask chunk.
"""

import sys

sys.path.insert(0, "/opt/trn_rl_repo")

import numpy as np

DIMS = [256, 128, 256, 512, 256, 128, 64, 32, 16, 10]
NCORES = 8
BATCH = 65536
SHARD = BATCH // NCORES  # 8192
MSUB = 512               # PSUM sub-tile columns
BLK = 2048               # block columns
NBLK = SHARD // BLK      # 4
NSUB = BLK // MSUB       # 4

# pack chunk layout (each chunk = 128 partitions x 8192 cols, fp16):
#   0,1: xT        2: m1        3,4: m2      5-8: m3
#   9,10: m4       11: m5       12: m6/m7/m8 partition-packed at rows 0/64/96
NPACK = 13

_PROG = {}


def _raise_sbuf_cap():
    # tile_utils.max_sbuf_usage is a stale 192KB constant; cayman has 208KB usable.
    import concourse.tile_utils as tu

    if getattr(tu, "max_sbuf_usage", 0) < 206 * 1024:
        tu.max_sbuf_usage = 206 * 1024


def _dedup_ldweights(nc):
    """Remove back-to-back redundant LDWEIGHTS (same stationary operand) so
    consecutive same-weight matmuls pipeline on the PE. Only drops LDW
    instructions that carry no semaphore waits/updates."""
    removed = 0
    for fn in nc.m.functions:
        for blk in fn.blocks:
            il = blk.instructions
            keep, last_sig = [], None
            for inst in il:
                nm = type(inst).__name__
                if nm == "InstLdweights":
                    sig = (str(inst.ins[0]), str(inst.is_transpose), str(inst.perf_mode),
                           str(getattr(inst, "tile_position", None)))
                    si = inst.sync_info
                    clean = si is None or (not si.on_wait and not si.on_update)
                    if sig == last_sig and clean:
                        removed += 1
                        continue
                    last_sig = sig
                keep.append(inst)
            if removed and len(keep) != len(il):
                while il:
                    il.pop()
                il.extend(keep)
    return removed


def _build_program():
    import concourse.bass as bass
    import concourse.tile as tile
    from concourse import bacc, mybir

    _raise_sbuf_cap()

    f16 = mybir.dt.float16
    f32 = mybir.dt.float32
    RELU = mybir.ActivationFunctionType.Relu
    IDENT = mybir.ActivationFunctionType.Identity
    ADD = mybir.AluOpType.add
    MAX = mybir.AluOpType.max

    nc = bacc.Bacc("TRN2", target_bir_lowering=False, debug=False, num_devices=NCORES)

    pack_d = nc.dram_tensor("pack", [128, NPACK, SHARD], f16, kind="ExternalInput").ap()
    w_d = {}
    w_d[1] = nc.dram_tensor("W1", [128, 2, 128], f16, kind="ExternalInput").ap()
    w_d[2] = nc.dram_tensor("W2", [128, 1, 256], f16, kind="ExternalInput").ap()
    w_d[3] = nc.dram_tensor("W3", [128, 2, 512], f16, kind="ExternalInput").ap()
    w_d[4] = nc.dram_tensor("W4", [128, 4, 256], f16, kind="ExternalInput").ap()
    w_d[5] = nc.dram_tensor("W5", [128, 2, 128], f16, kind="ExternalInput").ap()
    w_d[6] = nc.dram_tensor("W6", [128, 1, 64], f16, kind="ExternalInput").ap()
    w_d[7] = nc.dram_tensor("W7", [64, 32], f16, kind="ExternalInput").ap()
    w_d[8] = nc.dram_tensor("W8", [32, 16], f16, kind="ExternalInput").ap()
    w_d[9] = nc.dram_tensor("W9", [16, 10], f16, kind="ExternalInput").ap()
    b15_d = nc.dram_tensor("B15", [128, 10], f32, kind="ExternalInput").ap()
    b678_d = nc.dram_tensor("B678", [128, 1], f32, kind="ExternalInput").ap()
    b9_d = nc.dram_tensor("B9", [10, 1], f32, kind="ExternalInput").ap()
    out_d = nc.dram_tensor("outT", [10, SHARD], f32, kind="ExternalOutput").ap()

    with tile.TileContext(nc) as tc:
        with (
            tc.tile_pool(name="wpool", bufs=1) as wp,
            tc.tile_pool(name="mk", bufs=2) as mkp,
            tc.tile_pool(name="hr", bufs=1) as hrp,
            tc.tile_pool(name="hm", bufs=1) as hmp,
            tc.tile_pool(name="osb", bufs=2) as outp,
            tc.tile_pool(name="ps", bufs=6, space="PSUM") as psp,
            tc.tile_pool(name="ps678", bufs=1, space="PSUM") as ps678p,
        ):
            w1 = wp.tile([128, 2, 128], f16, tag="w1")
            w2 = wp.tile([128, 1, 256], f16, tag="w2")
            w3 = wp.tile([128, 2, 512], f16, tag="w3")
            w4 = wp.tile([128, 4, 256], f16, tag="w4")
            w5 = wp.tile([128, 2, 128], f16, tag="w5")
            w6 = wp.tile([128, 1, 64], f16, tag="w6")
            w789 = wp.tile([128, 64], f16, tag="w789")
            b15 = wp.tile([128, 10], f32, tag="b15")
            b678 = wp.tile([128, 1], f32, tag="b678")
            b9 = wp.tile([10, 1], f32, tag="b9")
            for t_, d_ in ((w1, w_d[1]), (w2, w_d[2]), (w3, w_d[3]), (w4, w_d[4]),
                           (w5, w_d[5]), (w6, w_d[6]), (b15, b15_d), (b678, b678_d), (b9, b9_d)):
                nc.sync.dma_start(t_[:], d_[:])
            nc.sync.dma_start(w789[0:64, 0:32], w_d[7][:])
            nc.sync.dma_start(w789[64:96, 32:48], w_d[8][:])
            nc.sync.dma_start(w789[96:112, 48:58], w_d[9][:])

            for b in range(NBLK):
                bs = bass.ts(b, BLK)
                pkx = mkp.tile([128, 2, BLK], f16, tag="pkx")
                m1 = mkp.tile([128, 1, BLK], f16, tag="m1")
                m2 = mkp.tile([128, 2, BLK], f16, tag="m2")
                m3 = mkp.tile([128, 4, BLK], f16, tag="m3", bufs=1)
                m4 = mkp.tile([128, 2, BLK], f16, tag="m4")
                m5 = mkp.tile([128, 1, BLK], f16, tag="m5")
                m678 = mkp.tile([128, 1, BLK], f16, tag="m678")
                nc.sync.dma_start(pkx[:], pack_d[:, 0:2, bs])
                nc.sync.dma_start(m1[:], pack_d[:, 2:3, bs])
                nc.sync.dma_start(m2[:], pack_d[:, 3:5, bs])
                nc.sync.dma_start(m3[:], pack_d[:, 5:9, bs])
                nc.sync.dma_start(m4[:], pack_d[:, 9:11, bs])
                nc.sync.dma_start(m5[:], pack_d[:, 11:12, bs])
                nc.sync.dma_start(m678[:], pack_d[:, 12:13, bs])

                def sub(ap3, c, t):
                    return ap3[:, c, bass.ts(t, MSUB)]

                def drain_relu(eng, dst, zsrc, bias_ap):
                    if eng == "act":
                        nc.scalar.activation(dst, zsrc, RELU, bias=bias_ap)
                    else:
                        nc.vector.tensor_scalar(dst, zsrc, bias_ap, 0.0, ADD, MAX)

                def mask_mul(eng, dst, src, msrc):
                    if eng == "gps":
                        nc.gpsimd.tensor_mul(dst, src, msrc)
                    else:
                        nc.vector.tensor_mul(dst, src, msrc)

                dr_i = [0]

                def pick_drain():
                    i = dr_i[0]
                    dr_i[0] += 1
                    return "act" if (i * 3) % 5 < 3 else "dve"

                # big layers, weight-major inner order: per (c,k) one LDW, NSUB
                # back-to-back matmuls
                hin = pkx
                layer_cfg = [
                    (2, w1, 1, m1, 0, "hr1", "hm1"),
                    (1, w2, 2, m2, 1, "hr2", "hm2"),
                    (2, w3, 4, m3, 3, "hr3", "hm3"),
                    (4, w4, 2, m4, 7, "hr4", "hm4"),
                    (2, w5, 1, m5, 9, "hr5", "hm5"),
                ]
                for (Kc, wt, Cc, mt, boff, hrtag, hmtag) in layer_cfg:
                    hr = hrp.tile([128, Cc, BLK], f16, tag=hrtag, name=hrtag + f"_{b}")
                    hm = hmp.tile([128, Cc, BLK], f16, tag=hmtag, name=hmtag + f"_{b}")
                    zs = {}
                    for c in range(Cc):
                        for t in range(NSUB):
                            zs[c, t] = psp.tile([128, MSUB], f32, tag="ps",
                                                name=f"z_{hrtag}_{b}_{c}_{t}")
                    for c in range(Cc):
                        for k in range(Kc):
                            wap = wt[:, k, bass.ts(c, 128)] if Cc > 1 else wt[:, k, :]
                            for t in range(NSUB):
                                nc.tensor.matmul(zs[c, t][:], wap, sub(hin, k, t),
                                                 start=(k == 0), stop=(k == Kc - 1))
                    for t in range(NSUB):
                        for c in range(Cc):
                            drain_relu(pick_drain(), sub(hr, c, t), zs[c, t][:],
                                       b15[:, boff + c:boff + c + 1])
                        meng = "gps" if (Cc == 1 and t % 2 == 1) else "dve"
                        mask_mul(meng, hr[:, :, bass.ts(t, MSUB)] if False else hm[:, :, bass.ts(t, MSUB)],
                                 hr[:, :, bass.ts(t, MSUB)], mt[:, :, bass.ts(t, MSUB)])
                    hin = hm

                hm5 = hin

                # ---- L6/L7/L8 partition-packed; PSUM per 1024-col half ----
                hr678 = hrp.tile([128, 1, BLK], f16, tag="hr678")
                hm678 = hmp.tile([128, 1, BLK], f16, tag="hm678")
                zh = [ps678p.tile([128, 2 * MSUB], f32, tag="z678", name=f"z678_{b}_{h}")
                      for h in range(NSUB // 2)]

                def ladder(prange, wap, tile_pos, brange):
                    p0, p1 = prange
                    for t in range(NSUB):
                        half, off = zh[t // 2], (t % 2) * MSUB
                        ts_ = bass.ts(t, MSUB)
                        rhs = (sub(hm5, 0, t) if p0 == 0 else
                               hm678[brange[0]:brange[1], 0, ts_])
                        if tile_pos is None:
                            nc.tensor.matmul(half[p0:p1, off:off + MSUB], wap, rhs,
                                             start=True, stop=True)
                        else:
                            nc.tensor.matmul(half[p0:p1, off:off + MSUB], wap, rhs,
                                             start=True, stop=True, tile_position=tile_pos)
                    for t in range(NSUB):
                        half, off = zh[t // 2], (t % 2) * MSUB
                        ts_ = bass.ts(t, MSUB)
                        drain_relu("dve" if t % 2 == 0 else "act",
                                   hr678[p0:p1, 0, ts_], half[p0:p1, off:off + MSUB],
                                   b678[p0:p1, 0:1])
                        mask_mul("dve" if t % 2 == 0 else "gps",
                                 hm678[p0:p1, 0, ts_], hr678[p0:p1, 0, ts_],
                                 m678[p0:p1, 0, ts_])

                ladder((0, 64), w6[:, 0, :], None, None)
                ladder((64, 96), w789[0:64, 0:32], (0, 64), (0, 64))
                ladder((96, 112), w789[64:96, 32:48], (64, 96), (64, 96))

                # ---- L9: 16 -> 10, bias only ----
                osb = outp.tile([10, BLK], f32, tag="osb", bufs=1)
                for t in range(NSUB):
                    z9 = psp.tile([128, MSUB], f32, tag="ps", name=f"z9_{b}_{t}")
                    nc.tensor.matmul(z9[0:10, :], w789[96:112, 48:58],
                                     hm678[96:112, 0, bass.ts(t, MSUB)],
                                     start=True, stop=True, tile_position=(96, 0))
                    nc.scalar.activation(osb[:, bass.ts(t, MSUB)], z9[0:10, :], IDENT, bias=b9[:, 0:1])
                nc.sync.dma_start(out_d[:, bs], osb[:])

    _dedup_ldweights(nc)
    nc.compile()
    return nc


def _get_program():
    if "nc" not in _PROG:
        _PROG["nc"] = _build_program()
    return _PROG["nc"]


def _host_prep(inputs):
    """Build per-core input maps (numpy only)."""
    x = np.asarray(inputs["x"], dtype=np.float32)
    Ws = [np.asarray(inputs[f"W{i}"], dtype=np.float32) for i in range(1, 10)]
    bs = [np.asarray(inputs[f"b{i}"], dtype=np.float32) for i in range(1, 10)]
    ms = [np.asarray(inputs[f"m{i}"], dtype=np.float32) for i in range(1, 9)]

    # fold dropout scale into next layer's weights; binarize masks
    Wf = [Ws[0]]
    for i in range(1, 9):
        s = float(ms[i - 1].max())
        if s <= 0.0:  # degenerate all-dropped mask; keep weights unscaled
            s = 1.0
        Wf.append(Ws[i] * np.float32(s))

    def pack_w(W, parts):
        K, N = W.shape
        Kc = (K + 127) // 128
        out = np.zeros((parts, Kc, N), dtype=np.float16)
        for k in range(Kc):
            blk = W[k * 128:(k + 1) * 128]
            out[: blk.shape[0], k, :] = blk.astype(np.float16)
        return out

    shared = {
        "W1": pack_w(Wf[0], 128), "W2": pack_w(Wf[1], 128), "W3": pack_w(Wf[2], 128),
        "W4": pack_w(Wf[3], 128), "W5": pack_w(Wf[4], 128), "W6": pack_w(Wf[5], 128),
        "W7": Wf[6].astype(np.float16), "W8": Wf[7].astype(np.float16),
        "W9": Wf[8].astype(np.float16),
    }
    b15 = np.zeros((128, 10), dtype=np.float32)
    b15[:, 0] = bs[0]
    b15[:, 1], b15[:, 2] = bs[1][0:128], bs[1][128:256]
    for c in range(4):
        b15[:, 3 + c] = bs[2][c * 128:(c + 1) * 128]
    b15[:, 7], b15[:, 8] = bs[3][0:128], bs[3][128:256]
    b15[:, 9] = bs[4]
    b678 = np.zeros((128, 1), dtype=np.float32)
    b678[0:64, 0], b678[64:96, 0], b678[96:112, 0] = bs[5], bs[6], bs[7]
    shared["B15"], shared["B678"] = b15, b678
    shared["B9"] = bs[8].reshape(10, 1)

    in_maps = []
    for c in range(NCORES):
        sl = slice(c * SHARD, (c + 1) * SHARD)
        pack = np.zeros((128, NPACK, SHARD), dtype=np.float16)
        xT = x[sl].T  # (256, SHARD)
        pack[:, 0, :] = xT[0:128]
        pack[:, 1, :] = xT[128:256]
        mT = [None] + [(ms[i][sl] != 0).T.astype(np.float16) for i in range(8)]  # 1-indexed
        pack[:, 2, :] = mT[1]
        pack[:, 3, :], pack[:, 4, :] = mT[2][0:128], mT[2][128:256]
        for k in range(4):
            pack[:, 5 + k, :] = mT[3][k * 128:(k + 1) * 128]
        pack[:, 9, :], pack[:, 10, :] = mT[4][0:128], mT[4][128:256]
        pack[:, 11, :] = mT[5]
        pack[0:64, 12, :] = mT[6]
        pack[64:96, 12, :] = mT[7]
        pack[96:112, 12, :] = mT[8]
        in_maps.append({"pack": pack, **shared})
    return in_maps


def kernel(**inputs) -> np.ndarray:
    from concourse.bass_utils import run_bass_kernel_spmd

    nc = _get_program()
    in_maps = _host_prep(inputs)
    res = run_bass_kernel_spmd(nc, in_maps, list(range(NCORES)))
    out = np.empty((BATCH, DIMS[-1]), dtype=np.float32)
    for c in range(NCORES):
        out[c * SHARD:(c + 1) * SHARD, :] = res.results[c]["outT"].T
    return out
